# revision 1
# baseline (speedup 1.0000x reference)
"""Causal self-attention (B=2, L=2048, D=1024, H=16) on 8 Trainium2 NeuronCores.

Sharding: batch x head-group data/tensor parallel. Core c handles batch
c//4 and heads 4*(c%4)..4*(c%4)+3. w_qkv is column-sharded by head,
w_proj row-sharded; the output all-reduce (sum of per-core partials
within each batch group) is done on the host after the gather.

Per-core pipeline, streamed per 512-row q supertile so ScalarE's exp
work overlaps the projection matmuls of neighboring supertiles:
  phase 1(qc): qT/kT = (x @ Wq|Wk)^T for q rows [512qc, 512qc+512)
  phase 2(qc): v = x @ Wv for the same rows (+ ones col for softmax sums)
  phase 3(qc): per head: S^T = k_chunk @ q^T (f32r), causal mask added as
        a bf16 PE matmul (identity @ triangular NEG boundary tile) into
        the same PSUM accumulation, exp on ScalarE (scale=1/8, bf16 out),
        P@V (bf16) with an appended ones row accumulating [o'; sums],
        normalize via approx-reciprocal + gpsimd partition broadcast
  phase 4(qc): out_partial = o_heads @ w_proj_local, streamed to DRAM
"""
import os
import sys
from contextlib import ExitStack

for _p in ("/opt/trn_rl_repo", "/root/.axon_site/_ro/trn_rl_repo"):
    if os.path.isdir(_p) and _p not in sys.path:
        sys.path.append(_p)

import numpy as np

import concourse.bass as bass  # noqa: F401
import concourse.mybir as mybir
import concourse.tile as tile
from concourse import bacc
from concourse.bass_utils import run_bass_kernel_spmd

dt = mybir.dt
F32, F32R, BF16 = dt.float32, dt.float32r, dt.bfloat16
EXP = mybir.ActivationFunctionType.Exp
GE = mybir.AluOpType.is_ge

L = 2048          # sequence length
D = 1024          # model dim
DH = 64           # head dim
HL = 4            # local heads per core
DC = D // 128     # 8 contraction chunks of 128
NQC = L // 512    # 4 q supertiles
NQB = L // 128    # 16 q blocks
N_CORES = 8
NEG = -1.0e5      # causal mask additive value (exp(NEG/8) == 0)

_nc_cache = {}


def _emit_body(nc, tc, rep, parts="1234"):
    r = f"r{rep}"
    xT, wqk, wv, wp, out = (nc.m_dram[k] for k in
                            ("xT", "wqk", "wv", "wp", "out"))

    with ExitStack() as ctx:
        const = ctx.enter_context(tc.tile_pool(name=f"const{r}", bufs=1))
        xT_sb = const.tile([128, DC * L], F32R)
        wqk_sb = const.tile([128, DC * 512], F32R)
        wv_sb = const.tile([128, DC * 256], F32R)
        wp_sb = const.tile([128, 2 * D], F32R)
        qkT_sb = const.tile([128, 4 * L], F32R)   # [qT01|qT23|kT01|kT23]
        v_sb = const.tile([128, NQB * HL, DH + 1], BF16)
        oT01 = const.tile([128, L], F32R)
        oT23 = const.tile([128, L], F32R)
        ident = const.tile([128, 128], BF16)
        trimask = const.tile([128, 128], BF16)  # 0 if j>=p else NEG

        # DMA order: wqk + xT[qt=0] interleaved (unblock phase 1 asap),
        # then wv (phase 2), remaining xT slices, wp (phase 4)
        for c in range(DC):
            nc.sync.dma_start(out=wqk_sb[:, c * 512:(c + 1) * 512],
                              in_=wqk[c * 128:(c + 1) * 128, :])
            nc.sync.dma_start(out=xT_sb[:, c * L: c * L + 512],
                              in_=xT[c * 128:(c + 1) * 128, 0:512])
        for c in range(DC):
            nc.sync.dma_start(out=wv_sb[:, c * 256:(c + 1) * 256],
                              in_=wv[c * 128:(c + 1) * 128, :])
        for qt in range(1, NQC):
            for c in range(DC):
                nc.sync.dma_start(
                    out=xT_sb[:, c * L + qt * 512: c * L + qt * 512 + 512],
                    in_=xT[c * 128:(c + 1) * 128, qt * 512:(qt + 1) * 512])
        nc.sync.dma_start(out=wp_sb[:], in_=wp[:])
        nc.gpsimd.memset(v_sb[:, :, DH:DH + 1], 1.0)

        # identity + boundary triangular mask (bf16) for the mask-add matmul
        nc.gpsimd.memset(ident[:], 0.0)
        nc.gpsimd.affine_select(out=ident[:], in_=ident[:],
                                compare_op=mybir.AluOpType.not_equal,
                                fill=1.0, base=0, pattern=[[-1, 128]],
                                channel_multiplier=1)
        nc.gpsimd.memset(trimask[:], 0.0)
        nc.gpsimd.affine_select(out=trimask[:], in_=trimask[:],
                                compare_op=GE, fill=NEG, base=0,
                                pattern=[[1, 128]], channel_multiplier=-1)

        # shared PSUM pools for the whole streamed loop (8 banks total):
        # pa: phase 1/2/4 accumulators (2) | ps_s: S^T pairs (4) | ps_o (2)
        pa = ctx.enter_context(tc.tile_pool(name=f"pa{r}", bufs=2,
                                            space="PSUM"))
        ps_s = ctx.enter_context(tc.tile_pool(name=f"ps_s{r}", bufs=2,
                                              space="PSUM"))
        ps_o = ctx.enter_context(tc.tile_pool(name=f"ps_o{r}", bufs=2,
                                              space="PSUM"))
        expp = ctx.enter_context(tc.tile_pool(name=f"expp{r}", bufs=3))
        rp = ctx.enter_context(tc.tile_pool(name=f"rp{r}", bufs=2))
        outp = ctx.enter_context(tc.tile_pool(name=f"outp{r}", bufs=3))

        def phase4(qc):
            """Projection for supertile qc's 4 q blocks."""
            for qb in range(4 * qc, 4 * qc + 4):
                for nh in range(2):
                    pf = pa.tile([128, 512], F32, tag="pa")
                    nc.tensor.matmul(pf[:], oT01[:, qb * 128:(qb + 1) * 128],
                                     wp_sb[:, nh * 512: nh * 512 + 512],
                                     start=True, stop=False)
                    nc.tensor.matmul(pf[:], oT23[:, qb * 128:(qb + 1) * 128],
                                     wp_sb[:, D + nh * 512: D + nh * 512 + 512],
                                     start=False, stop=True)
                    ot = outp.tile([128, 512], F32)
                    nc.vector.tensor_copy(ot[:], pf[:])
                    nc.sync.dma_start(
                        out=out[qb * 128:(qb + 1) * 128,
                                nh * 512:(nh + 1) * 512],
                        in_=ot[:])

        for qc in range(NQC):
            # ---- phase 1: qkT blocks for q columns of this supertile
            for cb in range(4):
                pt = pa.tile([128, 512], F32, tag="pa")
                for c in range(DC):
                    nc.tensor.matmul(
                        pt[:],
                        wqk_sb[:, c * 512 + cb * 128: c * 512 + (cb + 1) * 128],
                        xT_sb[:, c * L + qc * 512: c * L + qc * 512 + 512],
                        start=(c == 0), stop=(c == DC - 1))
                nc.vector.tensor_copy(
                    qkT_sb[:, cb * L + qc * 512: cb * L + qc * 512 + 512],
                    pt[:])

            # ---- phase 2: v rows for this supertile (4 q blocks)
            for qb in range(4 * qc, 4 * qc + 4):
                pt = pa.tile([128, 512], F32, tag="pa")
                for c in range(DC):
                    nc.tensor.matmul(
                        pt[:, 0:256],
                        xT_sb[:, c * L + qb * 128: c * L + (qb + 1) * 128],
                        wv_sb[:, c * 256:(c + 1) * 256],
                        start=(c == 0), stop=(c == DC - 1))
                nc.vector.tensor_copy(
                    v_sb[:, qb * HL:(qb + 1) * HL, 0:DH],
                    pt[:, 0:256].rearrange("p (h d) -> p h d", d=DH))

            # ---- phase 4 of the previous supertile: independent PE work
            # that overlaps ScalarE draining this supertile's exp queue
            if qc > 0 and "4" in parts:
                phase4(qc - 1)

            # ---- phase 3: attention for this supertile, per head PAIR.
            # The two heads of a pair live at partitions 0-63 / 64-127, so
            # their K=64 S^T matmuls run concurrently in distinct PE row
            # groups (auto tile_position from base_partition).
            nkc = 4 * qc + 4
            for hp in range(2 if "3" in parts else 0):
                qT_off = hp * L + qc * 512
                kT_off = (2 + hp) * L
                po0 = ps_o.tile([128, 512], F32, tag="po")
                po1 = ps_o.tile([128, 512], F32, tag="po")
                po = [po0, po1]

                def s_group(kc):
                    """Pair of concurrent S^T matmuls (+ bf16 boundary
                    mask-adds on diagonal chunks) + one exp."""
                    st = ps_s.tile([128, 1024], F32, tag="st")
                    et = expp.tile([128, 1024], BF16, tag="et")
                    m = 128 * kc - 512 * qc  # >= 0 on diagonal chunks
                    # skip fully-masked leading columns where f32r still
                    # runs at full rate (needs N >= 256)
                    s0 = m if m in (128, 256) else 0
                    for hh in range(2):
                        pb = 64 * hh
                        nc.tensor.matmul(
                            st[:, hh * 512 + s0:(hh + 1) * 512],
                            qkT_sb[pb:pb + 64,
                                   kT_off + kc * 128: kT_off + (kc + 1) * 128],
                            qkT_sb[pb:pb + 64, qT_off + s0: qT_off + 512],
                            start=True, stop=(m < 0))
                    if m >= 0:
                        for hh in range(2):
                            nc.tensor.matmul(
                                st[:, hh * 512 + m: hh * 512 + m + 128],
                                ident[:], trimask[:],
                                start=False, stop=True)
                    nc.scalar.activation(et[:], st[:], EXP, scale=0.125)
                    return et

                def pv(kc, et):
                    m = max(0, 128 * kc - 512 * qc)
                    for hh in range(2):
                        nc.tensor.matmul(
                            po[hh][0:DH + 1, m:512],
                            v_sb[:, kc * HL + 2 * hp + hh, :],
                            et[:, hh * 512 + m:(hh + 1) * 512],
                            start=(kc == 0), stop=(kc == nkc - 1))

                # software-pipelined: S(kc+1) overlaps exp/PV of chunk kc
                prev = None
                for kc in range(nkc):
                    et = s_group(kc)
                    if prev is not None:
                        pv(kc - 1, prev)
                    prev = et
                pv(nkc - 1, prev)

                # normalize: oT[:, qc] = o' * (1/sums) broadcast over dh
                oT = oT01 if hp == 0 else oT23
                for hh in range(2):
                    pb = 64 * hh
                    rs = rp.tile([1, 512], F32, tag="rs")
                    r1 = rp.tile([1, 512], F32, tag="r1")
                    r64 = rp.tile([64, 512], F32, tag="r64")
                    nc.vector.tensor_copy(rs[:], po[hh][DH:DH + 1, :])
                    nc.vector.reciprocal_approx_fast(r1[:], rs[:])
                    nc.gpsimd.partition_broadcast(r64[:], r1[:])
                    nc.vector.tensor_mul(
                        oT[pb:pb + 64, qc * 512:(qc + 1) * 512],
                        po[hh][0:DH, :], r64[:])

        if "4" in parts:
            phase4(NQC - 1)


def _build(reps=1, parts="1234"):
    nc = bacc.Bacc("TRN2", debug=False, target_bir_lowering=False)
    nc.m_dram = {
        "xT": nc.dram_tensor("xT", [D, L], F32R, kind="ExternalInput").ap(),
        "wqk": nc.dram_tensor("wqk", [D, 512], F32R, kind="ExternalInput").ap(),
        "wv": nc.dram_tensor("wv", [D, 256], F32R, kind="ExternalInput").ap(),
        "wp": nc.dram_tensor("wp", [128, 2 * D], F32R,
                             kind="ExternalInput").ap(),
        "out": nc.dram_tensor("out", [L, D], F32, kind="ExternalOutput").ap(),
    }
    with tile.TileContext(nc) as tc:
        for rep in range(reps):
            _emit_body(nc, tc, rep, parts)
    nc.compile()
    return nc


def _get_nc(reps=1, parts="1234"):
    key = (reps, parts)
    if key not in _nc_cache:
        _nc_cache[key] = _build(reps, parts)
    return _nc_cache[key]


def make_in_maps(x, w_qkv, w_proj):
    """Host-side sharding: per-core input dict (all contiguous fp32)."""
    x = np.asarray(x, dtype=np.float32)
    w_qkv = np.asarray(w_qkv, dtype=np.float32)
    w_proj = np.asarray(w_proj, dtype=np.float32)
    in_maps = []
    for c in range(N_CORES):
        b = c // 4
        hb = 256 * (c % 4)  # column offset of this core's 4 heads
        xTc = np.ascontiguousarray(x[b].T)
        wqk_c = np.ascontiguousarray(np.concatenate(
            [w_qkv[:, hb:hb + 256], w_qkv[:, D + hb: D + hb + 256]], axis=1))
        wv_c = np.ascontiguousarray(w_qkv[:, 2 * D + hb: 2 * D + hb + 256])
        wpl = w_proj[hb:hb + 256, :]
        wp_c = np.ascontiguousarray(np.concatenate([wpl[0:128], wpl[128:256]],
                                                   axis=1))
        in_maps.append({"xT": xTc, "wqk": wqk_c, "wv": wv_c, "wp": wp_c})
    return in_maps


def combine_outputs(outs):
    """Sum per-core partials within each batch group (host all-reduce)."""
    o0 = outs[0] + outs[1] + outs[2] + outs[3]
    o1 = outs[4] + outs[5] + outs[6] + outs[7]
    return np.stack([o0, o1]).astype(np.float32)


def kernel(x, w_qkv, w_proj):
    nc = _get_nc()
    in_maps = make_in_maps(x, w_qkv, w_proj)
    res = run_bass_kernel_spmd(nc, in_maps, list(range(N_CORES)))
    return combine_outputs([r["out"] for r in res.results])



# revision 13
# speedup vs baseline: 1.1793x; 1.1793x over previous
"""Causal self-attention (B=2, L=2048, D=1024, H=16) on 8 Trainium2 NeuronCores.

Sharding: batch x head-group data/tensor parallel. Core c handles batch
c//4 and heads 4*(c%4)..4*(c%4)+3. w_qkv is column-sharded by head,
w_proj row-sharded; the output all-reduce (sum of per-core partials
within each batch group) is done on the host after the gather.

v2 over the f32r baseline:
  - all matmul operands bf16 (halves DMA + SBUF; rel err ~3e-3 vs 2e-2 gate)
  - full causal column-skip on diagonal S chunks (bf16 has no N>=256
    full-rate constraint, unlike f32r)
  - exp processes only live columns via a strided [128,2,cols] AP
  - merged DMAs: 7 input descriptors instead of 33, one output DMA per
    q block (HWDGE is a single-slot ~625ns/instruction resource)
  - phase-4 PSUM drains alternate Pool/DVE, reciprocal reads PSUM directly
  - phase 1/2/4 matmuls injected as fillers between attention kc-groups
    so PE stays busy while ScalarE drains the exp queue

Per-core pipeline, per 512-row q supertile:
  phase 1(qc): qT/kT = (x @ Wq|Wk)^T for q rows [512qc, 512qc+512)
  phase 2(qc): v = x @ Wv for the same rows (+ ones col for softmax sums)
  phase 3(qc): per head pair: S^T = k_chunk @ q^T (bf16, K=64, two heads
        in distinct PE row groups), causal mask added as a bf16 PE matmul
        (identity @ triangular NEG boundary tile) into the same PSUM
        accumulation, exp on ScalarE (scale=1/8, bf16 out), P@V (bf16)
        with an appended ones row accumulating [o'; sums], normalize via
        approx-reciprocal + gpsimd partition broadcast
  phase 4(qc): out_partial = o_heads @ w_proj_local, streamed to DRAM
"""
import os
import sys
from collections import deque
from contextlib import ExitStack

for _p in ("/opt/trn_rl_repo", "/root/.axon_site/_ro/trn_rl_repo"):
    if os.path.isdir(_p) and _p not in sys.path:
        sys.path.append(_p)

import numpy as np
import ml_dtypes

import concourse.bass as bass  # noqa: F401
import concourse.mybir as mybir
import concourse.tile as tile
from concourse import bacc
from concourse.bass_utils import run_bass_kernel_spmd

dt = mybir.dt
F32, BF16 = dt.float32, dt.bfloat16
EXP = mybir.ActivationFunctionType.Exp
GE = mybir.AluOpType.is_ge

L = 2048          # sequence length
D = 1024          # model dim
DH = 64           # head dim
HL = 4            # local heads per core
DC = D // 128     # 8 contraction chunks of 128
NQC = L // 512    # 4 q supertiles
NQB = L // 128    # 16 q blocks
N_CORES = 8
NEG = -1.0e5      # causal mask additive value (exp(NEG/8) == 0)

_nc_cache = {}


def _emit_body(nc, tc, rep, parts="1234"):
    r = f"r{rep}"
    xT, wqk, wv, wp, out = (nc.m_dram[k] for k in
                            ("xT", "wqk", "wv", "wp", "out"))

    with ExitStack() as ctx:
        const = ctx.enter_context(tc.tile_pool(name=f"const{r}", bufs=1))
        xT_sb = const.tile([128, DC, L], BF16)
        wqk_sb = const.tile([128, DC, 512], BF16)
        wv_sb = const.tile([128, DC, 256], BF16)
        wp_sb = const.tile([128, 2 * D], BF16)
        qkT_sb = const.tile([128, 4 * L], BF16)   # [qT01|qT23|kT01|kT23]
        v_sb = const.tile([128, NQB * HL, DH + 1], BF16)
        oT01 = const.tile([128, L], BF16)
        oT23 = const.tile([128, L], BF16)
        ident = const.tile([128, 128], BF16)
        trimask = const.tile([128, 128], BF16)  # 0 if j>=p else NEG

        # Merged DMAs (HWDGE costs ~625ns per instruction regardless of
        # size). wqk/xT[qt=0] interleaved in 2-chunk quarters so phase 1's
        # first matmuls unblock early; then wv (phase 2), the remaining
        # xT supertiles, and wp.
        for c0 in range(0, DC, 2):
            nc.sync.dma_start(
                out=wqk_sb[:, c0:c0 + 2, :],
                in_=wqk[c0 * 128:(c0 + 2) * 128, :].rearrange(
                    "(c p) w -> p c w", p=128))
            nc.sync.dma_start(
                out=xT_sb[:, c0:c0 + 2, 0:512],
                in_=xT[c0 * 128:(c0 + 2) * 128, 0:512].rearrange(
                    "(c p) w -> p c w", p=128))
        nc.sync.dma_start(
            out=wv_sb[:],
            in_=wv[:].rearrange("(c p) w -> p c w", p=128))
        for qt in range(1, NQC):
            nc.sync.dma_start(
                out=xT_sb[:, :, qt * 512:(qt + 1) * 512],
                in_=xT[:, qt * 512:(qt + 1) * 512].rearrange(
                    "(c p) w -> p c w", p=128))
        nc.sync.dma_start(out=wp_sb[:], in_=wp[:])
        nc.gpsimd.memset(v_sb[:, :, DH:DH + 1], 1.0)

        # identity + boundary triangular mask (bf16) for the mask-add matmul
        nc.gpsimd.memset(ident[:], 0.0)
        nc.gpsimd.affine_select(out=ident[:], in_=ident[:],
                                compare_op=mybir.AluOpType.not_equal,
                                fill=1.0, base=0, pattern=[[-1, 128]],
                                channel_multiplier=1)
        nc.gpsimd.memset(trimask[:], 0.0)
        nc.gpsimd.affine_select(out=trimask[:], in_=trimask[:],
                                compare_op=GE, fill=NEG, base=0,
                                pattern=[[1, 128]], channel_multiplier=-1)

        # shared PSUM pools (8 banks total):
        # pa: phase 1/2/4 accumulators (2) | ps_s: S^T pairs (4) | ps_o (2)
        pa = ctx.enter_context(tc.tile_pool(name=f"pa{r}", bufs=2,
                                            space="PSUM"))
        ps_s = ctx.enter_context(tc.tile_pool(name=f"ps_s{r}", bufs=2,
                                              space="PSUM"))
        ps_o = ctx.enter_context(tc.tile_pool(name=f"ps_o{r}", bufs=2,
                                              space="PSUM"))
        expp = ctx.enter_context(tc.tile_pool(name=f"expp{r}", bufs=3))
        rp = ctx.enter_context(tc.tile_pool(name=f"rp{r}", bufs=2))
        outp = ctx.enter_context(tc.tile_pool(name=f"outp{r}", bufs=2))

        # ---- filler units: small PE work chunks from phases 1/2/4 that
        # are injected between attention kc-groups (PE would otherwise
        # idle there while ScalarE drains the exp queue). Emission order
        # defines the dependency graph, so consumers call need(tag) to
        # force-pop prerequisite units first.
        fillq = deque()   # (closure, tag-or-None on a unit's last step)
        done = set()

        def pop_fill(n=1):
            for _ in range(n):
                if fillq:
                    f, tag = fillq.popleft()
                    f()
                    if tag is not None:
                        done.add(tag)

        def flush_fill():
            pop_fill(len(fillq))

        def need(tag):
            while tag not in done and fillq:
                pop_fill(1)

        def push_unit(steps, tag):
            fillq.extend((s, None) for s in steps[:-1])
            fillq.append((steps[-1], tag))

        def run_unit(steps, tag):
            for s in steps:
                s()
            done.add(tag)

        def ph1_steps(qc, cb):
            """qkT block cb for supertile qc: 8 accum matmuls + copy."""
            box = {}

            def mk(c):
                def step():
                    if c == 0:
                        box["pt"] = pa.tile([128, 512], F32, tag="pa",
                                            name=f"pt1_{qc}_{cb}")
                    nc.tensor.matmul(
                        box["pt"][:],
                        wqk_sb[:, c, cb * 128:(cb + 1) * 128],
                        xT_sb[:, c, qc * 512:(qc + 1) * 512],
                        start=(c == 0), stop=(c == DC - 1))
                    if c == DC - 1:
                        nc.vector.tensor_copy(
                            qkT_sb[:, cb * L + qc * 512:
                                   cb * L + qc * 512 + 512],
                            box["pt"][:])
                return step
            return [mk(c) for c in range(DC)]

        def ph2_steps(qb):
            """v rows for q block qb: 8 accum matmuls (2/step) + copy."""
            box = {}

            def mk(c0):
                def step():
                    if c0 == 0:
                        box["pt"] = pa.tile([128, 512], F32, tag="pa",
                                            name=f"pt2_{qb}")
                    for c in (c0, c0 + 1):
                        nc.tensor.matmul(
                            box["pt"][:, 0:256],
                            xT_sb[:, c, qb * 128:(qb + 1) * 128],
                            wv_sb[:, c, :],
                            start=(c == 0), stop=(c == DC - 1))
                    if c0 == DC - 2:
                        nc.vector.tensor_copy(
                            v_sb[:, qb * HL:(qb + 1) * HL, 0:DH],
                            box["pt"][:, 0:256].rearrange(
                                "p (h d) -> p h d", d=DH))
                return step
            return [mk(c0) for c0 in range(0, DC, 2)]

        def ph4_steps(qb):
            """Projection for q block qb: 2x2 matmuls, drains split
            across Pool/DVE, one merged output DMA."""
            box = {}

            def mk(nh):
                def s1():
                    if nh == 0:
                        box["ot"] = outp.tile([128, 1024], BF16,
                                              name=f"ot_{qb}")
                    box[nh] = pa.tile([128, 512], F32, tag="pa",
                                      name=f"pf_{qb}_{nh}")
                    nc.tensor.matmul(box[nh][:],
                                     oT01[:, qb * 128:(qb + 1) * 128],
                                     wp_sb[:, nh * 512: nh * 512 + 512],
                                     start=True, stop=False)

                def s2():
                    nc.tensor.matmul(box[nh][:],
                                     oT23[:, qb * 128:(qb + 1) * 128],
                                     wp_sb[:, D + nh * 512: D + nh * 512 + 512],
                                     start=False, stop=True)
                    if nh == 0:   # Pool can't read PSUM; split Act/DVE
                        nc.scalar.copy(box["ot"][:, 0:512], box[nh][:])
                    else:
                        nc.vector.tensor_copy(
                            box["ot"][:, 512:1024], box[nh][:])
                    if nh == 1:
                        nc.sync.dma_start(
                            out=out[qb * 128:(qb + 1) * 128, :],
                            in_=box["ot"][:])
                return [s1, s2]
            # s1 (oT01 half) before both s2s: the s2s wait on oT23, so
            # this order keeps PE fed while the normalize chain runs
            a, b = mk(0), mk(1)
            return [a[0], b[0], a[1], b[1]]

        def push_ph1(qc, cbs):
            if "1" in parts:
                for cb in cbs:
                    push_unit(ph1_steps(qc, cb), ("p1", qc, cb))

        def push_ph2(qc):
            if "2" in parts:
                for qb in range(4 * qc, 4 * qc + 4):
                    push_unit(ph2_steps(qb), ("p2", qb))

        def push_ph4(qc):
            if "4" in parts:
                for qb in range(4 * qc, 4 * qc + 4):
                    push_unit(ph4_steps(qb), ("p4", qb))

        def ph3(qc, hp):
            """Attention for supertile qc, head pair hp (heads at
            partitions 0-63 / 64-127 run in distinct PE row groups)."""
            nkc = 4 * qc + 4
            qT_off = hp * L + qc * 512
            kT_off = (2 + hp) * L
            po0 = ps_o.tile([128, 512], F32, tag="po")
            po1 = ps_o.tile([128, 512], F32, tag="po")
            po = [po0, po1]

            def s_group(kc):
                """Concurrent S^T matmul pair (+ bf16 boundary mask-adds
                on the diagonal chunk) + one strided exp."""
                st = ps_s.tile([128, 2, 512], F32, tag="st")
                et = expp.tile([128, 2, 512], BF16, tag="et")
                m = 128 * kc - 512 * qc  # >= 0 on diagonal chunks
                s0 = max(0, m)           # fully-masked leading columns
                for hh in range(2):
                    pb = 64 * hh
                    nc.tensor.matmul(
                        st[:, hh, s0:512],
                        qkT_sb[pb:pb + 64,
                               kT_off + kc * 128: kT_off + (kc + 1) * 128],
                        qkT_sb[pb:pb + 64, qT_off + s0: qT_off + 512],
                        start=True, stop=(m < 0))
                if m >= 0:
                    for hh in range(2):
                        nc.tensor.matmul(
                            st[:, hh, m:m + 128],
                            ident[:], trimask[:],
                            start=False, stop=True)
                if s0:
                    for hh in range(2):
                        nc.scalar.activation(et[:, hh, s0:512],
                                             st[:, hh, s0:512],
                                             EXP, scale=0.125)
                else:
                    nc.scalar.activation(et[:], st[:], EXP, scale=0.125)
                return et

            def pv(kc, et):
                m = max(0, 128 * kc - 512 * qc)
                for hh in range(2):
                    nc.tensor.matmul(
                        po[hh][0:DH + 1, m:512],
                        v_sb[:, kc * HL + 2 * hp + hh, :],
                        et[:, hh, m:512],
                        start=(kc == 0), stop=(kc == nkc - 1))

            # software-pipelined: fillers + S(kc+1) overlap exp/PV of kc.
            # No opportunistic fillers in the last two iterations: keeps
            # the engine queues clear so the normalize chain (recip ->
            # bcast -> mul) runs promptly and frees the po banks.
            need(("p1", qc, hp))          # qT for this head pair
            prev = None
            for kc in range(nkc):
                need(("p1", kc // 4, 2 + hp))   # kT supertile of chunk kc
                if prev is not None:
                    need(("p2", kc - 1))        # v chunk for pv below
                if kc < nkc - 2:
                    pop_fill(2 if len(fillq) >= 48 else 1)
                et = s_group(kc)
                if prev is not None:
                    pv(kc - 1, prev)
                prev = et
            need(("p2", nkc - 1))
            pv(nkc - 1, prev)

            # normalize: oT[:, qc] = o' * (1/sums) broadcast over dh
            oT = oT01 if hp == 0 else oT23
            for hh in range(2):
                pb = 64 * hh
                r1 = rp.tile([1, 512], F32, tag="r1")
                r64 = rp.tile([64, 512], F32, tag="r64")
                nc.vector.reciprocal_approx_fast(r1[:], po[hh][DH:DH + 1, :])
                nc.gpsimd.partition_broadcast(r64[:], r1[:])
                nc.vector.tensor_mul(
                    oT[pb:pb + 64, qc * 512:(qc + 1) * 512],
                    po[hh][0:DH, :], r64[:])

        # ---- prologue: the minimum phase 1/2 work for ph3(0, hp=0)
        run_unit(ph1_steps(0, 0), ("p1", 0, 0))
        run_unit(ph1_steps(0, 2), ("p1", 0, 2))
        run_unit(ph2_steps(0), ("p2", 0))
        for qb in (1, 2, 3):
            push_unit(ph2_steps(qb), ("p2", qb))
        push_ph1(0, (1, 3))          # needed by ph3(0, hp=1)

        for qc in range(NQC):
            if qc + 1 < NQC:
                push_ph1(qc + 1, (0, 2, 1, 3))
                push_ph2(qc + 1)
            if qc > 0:
                push_ph4(qc - 1)
            if "3" in parts:
                ph3(qc, 0)
                flush_fill() if qc == 0 else None
                ph3(qc, 1)
            flush_fill()
        push_ph4(NQC - 1)
        flush_fill()


def _build(reps=1, parts="1234"):
    nc = bacc.Bacc("TRN2", debug=False, target_bir_lowering=False)
    nc.m_dram = {
        "xT": nc.dram_tensor("xT", [D, L], BF16, kind="ExternalInput").ap(),
        "wqk": nc.dram_tensor("wqk", [D, 512], BF16,
                              kind="ExternalInput").ap(),
        "wv": nc.dram_tensor("wv", [D, 256], BF16, kind="ExternalInput").ap(),
        "wp": nc.dram_tensor("wp", [128, 2 * D], BF16,
                             kind="ExternalInput").ap(),
        "out": nc.dram_tensor("out", [L, D], BF16, kind="ExternalOutput").ap(),
    }
    with tile.TileContext(nc) as tc:
        for rep in range(reps):
            _emit_body(nc, tc, rep, parts)
    nc.compile()
    return nc


def _get_nc(reps=1, parts="1234"):
    key = (reps, parts)
    if key not in _nc_cache:
        _nc_cache[key] = _build(reps, parts)
    return _nc_cache[key]


def make_in_maps(x, w_qkv, w_proj):
    """Host-side sharding: per-core input dict (contiguous bf16)."""
    bf16 = ml_dtypes.bfloat16
    x = np.asarray(x, dtype=np.float32)
    w_qkv = np.asarray(w_qkv, dtype=np.float32)
    w_proj = np.asarray(w_proj, dtype=np.float32)
    in_maps = []
    for c in range(N_CORES):
        b = c // 4
        hb = 256 * (c % 4)  # column offset of this core's 4 heads
        xTc = np.ascontiguousarray(x[b].T.astype(bf16))
        wqk_c = np.ascontiguousarray(np.concatenate(
            [w_qkv[:, hb:hb + 256], w_qkv[:, D + hb: D + hb + 256]],
            axis=1).astype(bf16))
        wv_c = np.ascontiguousarray(
            w_qkv[:, 2 * D + hb: 2 * D + hb + 256].astype(bf16))
        wpl = w_proj[hb:hb + 256, :]
        wp_c = np.ascontiguousarray(np.concatenate(
            [wpl[0:128], wpl[128:256]], axis=1).astype(bf16))
        in_maps.append({"xT": xTc, "wqk": wqk_c, "wv": wv_c, "wp": wp_c})
    return in_maps


def combine_outputs(outs):
    """Sum per-core partials within each batch group (host all-reduce)."""
    outs = [np.asarray(o, dtype=np.float32) for o in outs]
    o0 = outs[0] + outs[1] + outs[2] + outs[3]
    o1 = outs[4] + outs[5] + outs[6] + outs[7]
    return np.stack([o0, o1]).astype(np.float32)


def kernel(x, w_qkv, w_proj):
    nc = _get_nc()
    in_maps = make_in_maps(x, w_qkv, w_proj)
    res = run_bass_kernel_spmd(nc, in_maps, list(range(N_CORES)))
    return combine_outputs([r["out"] for r in res.results])


# revision 14
# speedup vs baseline: 1.3824x; 1.1722x over previous
"""Causal self-attention (B=2, L=2048, D=1024, H=16) on 8 Trainium2 NeuronCores.

Sharding: batch x head-group data/tensor parallel. Core c handles batch
c//4 and heads 4*(c%4)..4*(c%4)+3. w_qkv is column-sharded by head,
w_proj row-sharded; the output all-reduce (sum of per-core partials
within each batch group) is done on the host after the gather.

v2 over the f32r baseline:
  - all matmul operands bf16 (halves DMA + SBUF; rel err ~3e-3 vs 2e-2 gate)
  - full causal column-skip on diagonal S chunks (bf16 has no N>=256
    full-rate constraint, unlike f32r)
  - exp processes only live columns via a strided [128,2,cols] AP
  - merged DMAs: 7 input descriptors instead of 33, one output DMA per
    q block (HWDGE is a single-slot ~625ns/instruction resource)
  - phase-4 PSUM drains alternate Pool/DVE, reciprocal reads PSUM directly
  - phase 1/2/4 matmuls injected as fillers between attention kc-groups
    so PE stays busy while ScalarE drains the exp queue

Per-core pipeline, per 512-row q supertile:
  phase 1(qc): qT/kT = (x @ Wq|Wk)^T for q rows [512qc, 512qc+512)
  phase 2(qc): v = x @ Wv for the same rows (+ ones col for softmax sums)
  phase 3(qc): per head pair: S^T = k_chunk @ q^T (bf16, K=64, two heads
        in distinct PE row groups), causal mask added as a bf16 PE matmul
        (identity @ triangular NEG boundary tile) into the same PSUM
        accumulation, exp on ScalarE (scale=1/8, bf16 out), P@V (bf16)
        with an appended ones row accumulating [o'; sums], normalize via
        approx-reciprocal + gpsimd partition broadcast
  phase 4(qc): out_partial = o_heads @ w_proj_local, streamed to DRAM
"""
import os
import sys
from collections import deque
from contextlib import ExitStack

for _p in ("/opt/trn_rl_repo", "/root/.axon_site/_ro/trn_rl_repo"):
    if os.path.isdir(_p) and _p not in sys.path:
        sys.path.append(_p)

import numpy as np
import ml_dtypes

import concourse.bass as bass  # noqa: F401
import concourse.mybir as mybir
import concourse.tile as tile
from concourse import bacc
from concourse.bass_utils import run_bass_kernel_spmd

dt = mybir.dt
F32, BF16 = dt.float32, dt.bfloat16
EXP = mybir.ActivationFunctionType.Exp
GE = mybir.AluOpType.is_ge

L = 2048          # sequence length
D = 1024          # model dim
DH = 64           # head dim
HL = 4            # local heads per core
DC = D // 128     # 8 contraction chunks of 128
NQC = L // 512    # 4 q supertiles
NQB = L // 128    # 16 q blocks
N_CORES = 8
NEG = -1.0e5      # causal mask additive value (exp(NEG/8) == 0)

_nc_cache = {}


def _emit_body(nc, tc, rep, parts="1234"):
    r = f"r{rep}"
    xT, wqk, wv, wp, out = (nc.m_dram[k] for k in
                            ("xT", "wqk", "wv", "wp", "out"))

    with ExitStack() as ctx:
        const = ctx.enter_context(tc.tile_pool(name=f"const{r}", bufs=1))
        xT_sb = const.tile([128, DC, L], BF16)
        wqk_sb = const.tile([128, DC, 512], BF16)
        wv_sb = const.tile([128, DC, 256], BF16)
        wp_sb = const.tile([128, 2 * D], BF16)
        qkT_sb = const.tile([128, 4 * L], BF16)   # [qT01|qT23|kT01|kT23]
        v_sb = const.tile([128, NQB * HL, DH + 1], BF16)
        oT01 = const.tile([128, L], BF16)
        oT23 = const.tile([128, L], BF16)
        ident = const.tile([128, 128], BF16)
        trimask = const.tile([128, 128], BF16)  # 0 if j>=p else NEG

        # Merged DMAs (HWDGE costs ~625ns per instruction regardless of
        # size). wqk/xT[qt=0] interleaved in 2-chunk quarters so phase 1's
        # first matmuls unblock early; then wv (phase 2), the remaining
        # xT supertiles, and wp.
        for c0 in range(0, DC, 2):
            nc.sync.dma_start(
                out=wqk_sb[:, c0:c0 + 2, :],
                in_=wqk[c0 * 128:(c0 + 2) * 128, :].rearrange(
                    "(c p) w -> p c w", p=128))
            nc.sync.dma_start(
                out=xT_sb[:, c0:c0 + 2, 0:512],
                in_=xT[c0 * 128:(c0 + 2) * 128, 0:512].rearrange(
                    "(c p) w -> p c w", p=128))
        nc.sync.dma_start(
            out=wv_sb[:],
            in_=wv[:].rearrange("(c p) w -> p c w", p=128))
        for qt in range(1, NQC):
            nc.sync.dma_start(
                out=xT_sb[:, :, qt * 512:(qt + 1) * 512],
                in_=xT[:, qt * 512:(qt + 1) * 512].rearrange(
                    "(c p) w -> p c w", p=128))
        nc.sync.dma_start(out=wp_sb[:], in_=wp[:])
        nc.gpsimd.memset(v_sb[:, :, DH:DH + 1], 1.0)

        # identity + boundary triangular mask (bf16) for the mask-add matmul
        nc.gpsimd.memset(ident[:], 0.0)
        nc.gpsimd.affine_select(out=ident[:], in_=ident[:],
                                compare_op=mybir.AluOpType.not_equal,
                                fill=1.0, base=0, pattern=[[-1, 128]],
                                channel_multiplier=1)
        nc.gpsimd.memset(trimask[:], 0.0)
        nc.gpsimd.affine_select(out=trimask[:], in_=trimask[:],
                                compare_op=GE, fill=NEG, base=0,
                                pattern=[[1, 128]], channel_multiplier=-1)

        # shared PSUM pools (8 banks total):
        # pa: phase 1/2/4 accumulators (2) | ps_s: S^T pairs (4) | ps_o (2)
        pa = ctx.enter_context(tc.tile_pool(name=f"pa{r}", bufs=2,
                                            space="PSUM"))
        ps_s = ctx.enter_context(tc.tile_pool(name=f"ps_s{r}", bufs=2,
                                              space="PSUM"))
        ps_o = ctx.enter_context(tc.tile_pool(name=f"ps_o{r}", bufs=2,
                                              space="PSUM"))
        expp = ctx.enter_context(tc.tile_pool(name=f"expp{r}", bufs=3))
        rp = ctx.enter_context(tc.tile_pool(name=f"rp{r}", bufs=2))
        outp = ctx.enter_context(tc.tile_pool(name=f"outp{r}", bufs=2))

        # ---- filler units: small PE work chunks from phases 1/2/4 that
        # are injected between attention kc-groups (PE would otherwise
        # idle there while ScalarE drains the exp queue). Emission order
        # defines the dependency graph, so consumers call need(tag) to
        # force-pop prerequisite units first.
        fillq = deque()   # (closure, tag-or-None on a unit's last step)
        done = set()

        def pop_fill(n=1):
            for _ in range(n):
                if fillq:
                    f, tag = fillq.popleft()
                    f()
                    if tag is not None:
                        done.add(tag)

        def flush_fill():
            pop_fill(len(fillq))

        def need(tag):
            while tag not in done and fillq:
                pop_fill(1)

        def push_unit(steps, tag):
            fillq.extend((s, None) for s in steps[:-1])
            fillq.append((steps[-1], tag))

        def run_unit(steps, tag):
            for s in steps:
                s()
            done.add(tag)

        def ph1_steps(qc, cb):
            """qkT block cb for supertile qc: 8 accum matmuls + copy."""
            box = {}

            def mk(c):
                def step():
                    if c == 0:
                        box["pt"] = pa.tile([128, 512], F32, tag="pa",
                                            name=f"pt1_{qc}_{cb}")
                    nc.tensor.matmul(
                        box["pt"][:],
                        wqk_sb[:, c, cb * 128:(cb + 1) * 128],
                        xT_sb[:, c, qc * 512:(qc + 1) * 512],
                        start=(c == 0), stop=(c == DC - 1))
                    if c == DC - 1:
                        nc.vector.tensor_copy(
                            qkT_sb[:, cb * L + qc * 512:
                                   cb * L + qc * 512 + 512],
                            box["pt"][:])
                return step
            return [mk(c) for c in range(DC)]

        def ph2_steps(qb):
            """v rows for q block qb: 8 accum matmuls (2/step) + copy."""
            box = {}

            def mk(c0):
                def step():
                    if c0 == 0:
                        box["pt"] = pa.tile([128, 512], F32, tag="pa",
                                            name=f"pt2_{qb}")
                    for c in (c0, c0 + 1):
                        nc.tensor.matmul(
                            box["pt"][:, 0:256],
                            xT_sb[:, c, qb * 128:(qb + 1) * 128],
                            wv_sb[:, c, :],
                            start=(c == 0), stop=(c == DC - 1))
                    if c0 == DC - 2:
                        nc.vector.tensor_copy(
                            v_sb[:, qb * HL:(qb + 1) * HL, 0:DH],
                            box["pt"][:, 0:256].rearrange(
                                "p (h d) -> p h d", d=DH))
                return step
            return [mk(c0) for c0 in range(0, DC, 2)]

        def ph4_steps(qb):
            """Projection for q block qb: 2x2 matmuls, drains split
            across Pool/DVE, one merged output DMA."""
            box = {}

            def mk(nh):
                def s1():
                    if nh == 0:
                        box["ot"] = outp.tile([128, 1024], BF16,
                                              name=f"ot_{qb}")
                    box[nh] = pa.tile([128, 512], F32, tag="pa",
                                      name=f"pf_{qb}_{nh}")
                    nc.tensor.matmul(box[nh][:],
                                     oT01[:, qb * 128:(qb + 1) * 128],
                                     wp_sb[:, nh * 512: nh * 512 + 512],
                                     start=True, stop=False)

                def s2():
                    nc.tensor.matmul(box[nh][:],
                                     oT23[:, qb * 128:(qb + 1) * 128],
                                     wp_sb[:, D + nh * 512: D + nh * 512 + 512],
                                     start=False, stop=True)
                    if nh == 0:   # Pool can't read PSUM; split Act/DVE
                        nc.scalar.copy(box["ot"][:, 0:512], box[nh][:])
                    else:
                        nc.vector.tensor_copy(
                            box["ot"][:, 512:1024], box[nh][:])
                    if nh == 1:
                        nc.sync.dma_start(
                            out=out[qb * 128:(qb + 1) * 128, :],
                            in_=box["ot"][:])
                return [s1, s2]
            # s1 (oT01 half) before both s2s: the s2s wait on oT23, so
            # this order keeps PE fed while the normalize chain runs
            a, b = mk(0), mk(1)
            return [a[0], b[0], a[1], b[1]]

        def push_ph1(qc, cbs):
            if "1" in parts:
                for cb in cbs:
                    push_unit(ph1_steps(qc, cb), ("p1", qc, cb))

        def push_ph2(qc):
            if "2" in parts:
                for qb in range(4 * qc, 4 * qc + 4):
                    push_unit(ph2_steps(qb), ("p2", qb))

        def push_ph4(qc):
            if "4" in parts:
                for qb in range(4 * qc, 4 * qc + 4):
                    push_unit(ph4_steps(qb), ("p4", qb))

        def ph3(qc, hp):
            """Attention for supertile qc, head pair hp (heads at
            partitions 0-63 / 64-127 run in distinct PE row groups)."""
            nkc = 4 * qc + 4
            qT_off = hp * L + qc * 512
            kT_off = (2 + hp) * L
            po0 = ps_o.tile([128, 512], F32, tag="po")
            po1 = ps_o.tile([128, 512], F32, tag="po")
            po = [po0, po1]

            def s_group(kc):
                """Concurrent S^T matmul pair (+ bf16 boundary mask-adds
                on the diagonal chunk) + one strided exp."""
                st = ps_s.tile([128, 2, 512], F32, tag="st")
                et = expp.tile([128, 2, 512], BF16, tag="et")
                m = 128 * kc - 512 * qc  # >= 0 on diagonal chunks
                s0 = max(0, m)           # fully-masked leading columns
                for hh in range(2):
                    pb = 64 * hh
                    nc.tensor.matmul(
                        st[:, hh, s0:512],
                        qkT_sb[pb:pb + 64,
                               kT_off + kc * 128: kT_off + (kc + 1) * 128],
                        qkT_sb[pb:pb + 64, qT_off + s0: qT_off + 512],
                        start=True, stop=(m < 0))
                if m >= 0:
                    for hh in range(2):
                        nc.tensor.matmul(
                            st[:, hh, m:m + 128],
                            ident[:], trimask[:],
                            start=False, stop=True)
                if s0:
                    for hh in range(2):
                        nc.scalar.activation(et[:, hh, s0:512],
                                             st[:, hh, s0:512],
                                             EXP, scale=0.125)
                else:
                    nc.scalar.activation(et[:], st[:], EXP, scale=0.125)
                return et

            def pv(kc, et):
                m = max(0, 128 * kc - 512 * qc)
                for hh in range(2):
                    nc.tensor.matmul(
                        po[hh][0:DH + 1, m:512],
                        v_sb[:, kc * HL + 2 * hp + hh, :],
                        et[:, hh, m:512],
                        start=(kc == 0), stop=(kc == nkc - 1))

            # software-pipelined: fillers + S(kc+1) overlap exp/PV of kc.
            # No opportunistic fillers in the last two iterations: keeps
            # the engine queues clear so the normalize chain (recip ->
            # bcast -> mul) runs promptly and frees the po banks.
            need(("p1", qc, hp))          # qT for this head pair
            prev = None
            for kc in range(nkc):
                need(("p1", kc // 4, 2 + hp))   # kT supertile of chunk kc
                if prev is not None:
                    need(("p2", kc - 1))        # v chunk for pv below
                if kc < nkc - 2:
                    pop_fill(2 if len(fillq) >= 48 else 1)
                et = s_group(kc)
                if prev is not None:
                    pv(kc - 1, prev)
                prev = et
            need(("p2", nkc - 1))
            pv(nkc - 1, prev)

            # normalize: oT[:, qc] = o' * (1/sums) broadcast over dh
            oT = oT01 if hp == 0 else oT23
            for hh in range(2):
                pb = 64 * hh
                rs = rp.tile([1, 512], F32, tag="rs")
                r1 = rp.tile([1, 512], F32, tag="r1")
                r64 = rp.tile([64, 512], F32, tag="r64")
                nc.vector.tensor_copy(rs[:], po[hh][DH:DH + 1, :])
                nc.vector.reciprocal_approx_fast(r1[:], rs[:])
                nc.gpsimd.partition_broadcast(r64[:], r1[:])
                nc.vector.tensor_mul(
                    oT[pb:pb + 64, qc * 512:(qc + 1) * 512],
                    po[hh][0:DH, :], r64[:])

        # ---- prologue: the minimum phase 1/2 work for ph3(0, hp=0)
        run_unit(ph1_steps(0, 0), ("p1", 0, 0))
        run_unit(ph1_steps(0, 2), ("p1", 0, 2))
        run_unit(ph2_steps(0), ("p2", 0))
        for qb in (1, 2, 3):
            push_unit(ph2_steps(qb), ("p2", qb))
        push_ph1(0, (1, 3))          # needed by ph3(0, hp=1)

        for qc in range(NQC):
            if qc + 1 < NQC:
                push_ph1(qc + 1, (0, 2, 1, 3))
                push_ph2(qc + 1)
            if qc > 0:
                push_ph4(qc - 1)
            if "3" in parts:
                ph3(qc, 0)
                flush_fill() if qc == 0 else None
                ph3(qc, 1)
            flush_fill()
        push_ph4(NQC - 1)
        flush_fill()


def _build(reps=1, parts="1234"):
    nc = bacc.Bacc("TRN2", debug=False, target_bir_lowering=False)
    nc.m_dram = {
        "xT": nc.dram_tensor("xT", [D, L], BF16, kind="ExternalInput").ap(),
        "wqk": nc.dram_tensor("wqk", [D, 512], BF16,
                              kind="ExternalInput").ap(),
        "wv": nc.dram_tensor("wv", [D, 256], BF16, kind="ExternalInput").ap(),
        "wp": nc.dram_tensor("wp", [128, 2 * D], BF16,
                             kind="ExternalInput").ap(),
        "out": nc.dram_tensor("out", [L, D], BF16, kind="ExternalOutput").ap(),
    }
    with tile.TileContext(nc) as tc:
        for rep in range(reps):
            _emit_body(nc, tc, rep, parts)
    nc.compile()
    return nc


def _get_nc(reps=1, parts="1234"):
    key = (reps, parts)
    if key not in _nc_cache:
        _nc_cache[key] = _build(reps, parts)
    return _nc_cache[key]


def make_in_maps(x, w_qkv, w_proj):
    """Host-side sharding: per-core input dict (contiguous bf16)."""
    bf16 = ml_dtypes.bfloat16
    x = np.asarray(x, dtype=np.float32)
    w_qkv = np.asarray(w_qkv, dtype=np.float32)
    w_proj = np.asarray(w_proj, dtype=np.float32)
    in_maps = []
    for c in range(N_CORES):
        b = c // 4
        hb = 256 * (c % 4)  # column offset of this core's 4 heads
        xTc = np.ascontiguousarray(x[b].T.astype(bf16))
        wqk_c = np.ascontiguousarray(np.concatenate(
            [w_qkv[:, hb:hb + 256], w_qkv[:, D + hb: D + hb + 256]],
            axis=1).astype(bf16))
        wv_c = np.ascontiguousarray(
            w_qkv[:, 2 * D + hb: 2 * D + hb + 256].astype(bf16))
        wpl = w_proj[hb:hb + 256, :]
        wp_c = np.ascontiguousarray(np.concatenate(
            [wpl[0:128], wpl[128:256]], axis=1).astype(bf16))
        in_maps.append({"xT": xTc, "wqk": wqk_c, "wv": wv_c, "wp": wp_c})
    return in_maps


def combine_outputs(outs):
    """Sum per-core partials within each batch group (host all-reduce)."""
    outs = [np.asarray(o, dtype=np.float32) for o in outs]
    o0 = outs[0] + outs[1] + outs[2] + outs[3]
    o1 = outs[4] + outs[5] + outs[6] + outs[7]
    return np.stack([o0, o1]).astype(np.float32)


def kernel(x, w_qkv, w_proj):
    nc = _get_nc()
    in_maps = make_in_maps(x, w_qkv, w_proj)
    res = run_bass_kernel_spmd(nc, in_maps, list(range(N_CORES)))
    return combine_outputs([r["out"] for r in res.results])
